# revision 6
# baseline (speedup 1.0000x reference)
"""Trainium2 Bass kernel for a single-head causal attention block.

Reference computation (per batch b):
    K = Xk @ WK ; V = Xv @ WV ; Q = Xq @ WQ           # [S, D]
    S_ = (Q @ K^T) / sqrt(D) + causal_mask            # [S, S]
    out = softmax(S_, axis=-1) @ V                    # [S, D]

Shapes: B=4, S=2048, D_IN=D_OUT=1024, fp32.

Sharding: 8 cores = (batch b, query-half h).  Core (b, h) computes the
attention output for queries [h*1024, (h+1)*1024) of batch b.  It projects
Q for its own 1024 query rows and K/V for all 2048 key rows of its batch
(K/V projection is duplicated across the 2 cores of a batch), then does a
2-pass softmax attention against an additive causal mask streamed from
DRAM (mask rows differ per core, so one homogeneous SPMD program serves
all 8 cores).

Matmul layout notes (out = lhsT.T @ rhs, contraction on partitions):
  - host passes Xq^T/Xk^T/Xv^T so projections read [din, seq] directly
  - Q and K are produced transposed ([dout, seq]) for the score matmuls
  - V is produced natural ([seq, dout]) for the PV matmuls
  - softmax probs are transposed on the PE (identity matmul) for PV
All matmuls run as float32r (1-pass fp22 mantissa) — the standard fp32
compute mode on TRN2.
"""

import numpy as np

import concourse.bass as bass
from concourse import bacc
import concourse.mybir as mybir
import concourse.tile as tile
from concourse.bass_utils import run_bass_kernel_spmd
from concourse.masks import make_identity

P = 128
B, S, DIN, DOUT = 4, 2048, 1024, 1024
QSH = 1024          # query rows per core
KO = DIN // P       # 8 contraction sub-tiles for the projections
DO = DOUT // P      # 8 dout sub-tiles
NT = S // P         # 16 key tiles
F32 = mybir.dt.float32
F32R = mybir.dt.float32r
SCALE = 1.0 / float(np.sqrt(DOUT))
NEG = -1.0e9

_NC_CACHE = {}


def _build_projection(nc, tc, ctx_pools, x3, w3, consume, seq, chunk, lhs_from_x):
    """Emit one projection pass.

    x3: [128, KO, seq] DRAM view of X^T, streamed in chunks along seq.
    w3: [128, KO, DOUT] SBUF-resident weight.
    consume(psum_ap, j0, j1): copy a finished [128, n] PSUM tile out.
    lhs_from_x: if True emit V-style (lhsT = x chunk of 128 rows, rhs = w,
      produces natural [seq, dout]); else Q/K-style (lhsT = w column block,
      rhs = x chunk, produces transposed [dout, seq]).
    """
    xpool, pspool = ctx_pools
    nchunks = seq // chunk
    for c in range(nchunks):
        x_sb = xpool.tile([P, KO, chunk], F32R, name="xstream")
        for o in range(KO):
            nc.sync.dma_start(
                out=x_sb[:, o, :], in_=x3[:, o, c * chunk : (c + 1) * chunk]
            )
        if lhs_from_x:
            # V path: for each 128-row slice of the chunk, out[seq128, dout512]
            for t in range(chunk // P):
                for dh in range(DOUT // 512):
                    ps = pspool.tile([P, 512], F32, name="proj_ps")
                    for k in range(KO):
                        nc.tensor.matmul(
                            ps,
                            lhsT=(x_sb[:, k, t * P : (t + 1) * P]),
                            rhs=(w3[:, k, dh * 512 : (dh + 1) * 512]),
                            start=(k == 0),
                            stop=(k == KO - 1),
                        )
                    consume(ps, c * chunk + t * P, dh * 512)
        else:
            # Q/K path: out[dout128, seq_chunk]
            for o in range(DO):
                ps = pspool.tile([P, chunk], F32, name="proj_ps")
                for k in range(KO):
                    nc.tensor.matmul(
                        ps,
                        lhsT=(w3[:, k, o * P : (o + 1) * P]),
                        rhs=(x_sb[:, k, :]),
                        start=(k == 0),
                        stop=(k == KO - 1),
                    )
                consume(ps, o, c * chunk)


def _build_bass():
    nc = bacc.Bacc()
    xqT = nc.declare_dram_parameter("xqT", [DIN, QSH], F32R, isOutput=False)
    xkT = nc.declare_dram_parameter("xkT", [DIN, S], F32R, isOutput=False)
    xvT = nc.declare_dram_parameter("xvT", [DIN, S], F32R, isOutput=False)
    wq = nc.declare_dram_parameter("wq", [DIN, DOUT], F32R, isOutput=False)
    wk = nc.declare_dram_parameter("wk", [DIN, DOUT], F32R, isOutput=False)
    wv = nc.declare_dram_parameter("wv", [DIN, DOUT], F32R, isOutput=False)
    mask = nc.declare_dram_parameter("mask", [QSH, S], F32, isOutput=False)
    out = nc.declare_dram_parameter("out", [QSH, DOUT], F32, isOutput=True)

    xq3 = xqT[:, :].rearrange("(o p) q -> p o q", p=P)
    xk3 = xkT[:, :].rearrange("(o p) s -> p o s", p=P)
    xv3 = xvT[:, :].rearrange("(o p) s -> p o s", p=P)
    wq3 = wq[:, :].rearrange("(o p) d -> p o d", p=P)
    wk3 = wk[:, :].rearrange("(o p) d -> p o d", p=P)
    wv3 = wv[:, :].rearrange("(o p) d -> p o d", p=P)

    with tile.TileContext(nc) as tc:
        with (
            tc.tile_pool(name="persist", bufs=1) as persist,
            tc.tile_pool(name="dram", bufs=1, space="DRAM") as dram,
        ):
            kt_sb = persist.tile([P, DO, S], F32R, name="kt")      # K^T  [dout, s]
            v_sb = persist.tile([P, NT, DOUT], F32R, name="v")     # V    [s, dout]
            # One DRAM scratch tile per dout block: keeps every spill DMA at
            # a single-writer / single-reader dependency (walrus caps the
            # number of sync waits one DMA descriptor can carry).
            qt_dram = [
                dram.tile([P, QSH], F32R, name=f"qt_scratch_{o}")
                for o in range(DO)
            ]

            # ---- Phase Q: Q^T = WQ^T @ Xq^T, scaled by 1/sqrt(d), spilled to DRAM
            with (
                tc.tile_pool(name="wpool_q", bufs=1) as wpool,
                tc.tile_pool(name="xs_q", bufs=2) as xpool,
                tc.tile_pool(name="ps_q", bufs=4, space="PSUM") as pspool,
                tc.tile_pool(name="qo", bufs=4) as qopool,
            ):
                w_sb = wpool.tile([P, KO, DOUT], F32R, name="w")
                for o in range(KO):
                    nc.sync.dma_start(out=w_sb[:, o, :], in_=wq3[:, o, :])

                def consume_q(ps, o, j):
                    o_sb = qopool.tile([P, 512], F32R, name="q_out")
                    nc.vector.tensor_scalar_mul(o_sb, ps, SCALE)
                    nc.sync.dma_start(
                        out=qt_dram[o][:, j : j + 512], in_=o_sb
                    )

                _build_projection(
                    nc, tc, (xpool, pspool), xq3, w_sb, consume_q,
                    seq=QSH, chunk=512, lhs_from_x=False,
                )

            # ---- Phase K: K^T = WK^T @ Xk^T, kept resident in SBUF
            with (
                tc.tile_pool(name="wpool_k", bufs=1) as wpool,
                tc.tile_pool(name="xs_k", bufs=2) as xpool,
                tc.tile_pool(name="ps_k", bufs=4, space="PSUM") as pspool,
            ):
                w_sb = wpool.tile([P, KO, DOUT], F32R, name="w")
                for o in range(KO):
                    nc.sync.dma_start(out=w_sb[:, o, :], in_=wk3[:, o, :])

                def consume_k(ps, o, j):
                    nc.vector.tensor_copy(kt_sb[:, o, j : j + 512], ps)

                _build_projection(
                    nc, tc, (xpool, pspool), xk3, w_sb, consume_k,
                    seq=S, chunk=512, lhs_from_x=False,
                )

            # ---- Phase V: V = Xv @ WV, kept resident in SBUF
            with (
                tc.tile_pool(name="wpool_v", bufs=1) as wpool,
                tc.tile_pool(name="xs_v", bufs=2) as xpool,
                tc.tile_pool(name="ps_v", bufs=4, space="PSUM") as pspool,
            ):
                w_sb = wpool.tile([P, KO, DOUT], F32R, name="w")
                for o in range(KO):
                    nc.sync.dma_start(out=w_sb[:, o, :], in_=wv3[:, o, :])

                def consume_v(ps, s0, d0):
                    nc.vector.tensor_copy(
                        v_sb[:, s0 // P, d0 : d0 + 512], ps
                    )

                _build_projection(
                    nc, tc, (xpool, pspool), xv3, w_sb, consume_v,
                    seq=S, chunk=256, lhs_from_x=True,
                )

            # ---- Phase A: attention, one 128-row query tile at a time
            with (
                tc.tile_pool(name="ident", bufs=1) as ipool,
                tc.tile_pool(name="qts", bufs=2) as qtpool,
                tc.tile_pool(name="msk", bufs=2) as mpool,
                tc.tile_pool(name="srow", bufs=2) as spool,
                tc.tile_pool(name="pt", bufs=1) as ptpool,
                tc.tile_pool(name="ao", bufs=2) as aopool,
                tc.tile_pool(name="stat", bufs=3) as stpool,
                tc.tile_pool(name="ps_s", bufs=2, space="PSUM") as psS,
                tc.tile_pool(name="ps_t", bufs=2, space="PSUM") as psT,
                tc.tile_pool(name="ps_o", bufs=2, space="PSUM") as psO,
            ):
                ident = ipool.tile([P, P], F32, name="identity")
                make_identity(nc, ident)

                for qt in range(QSH // P):
                    q_sb = qtpool.tile([P, DO, P], F32R, name="q_tile")
                    for o in range(DO):
                        nc.sync.dma_start(
                            out=q_sb[:, o, :],
                            in_=qt_dram[o][:, qt * P : (qt + 1) * P],
                        )
                    m_sb = mpool.tile([P, S], F32, name="mask_rows")
                    nc.sync.dma_start(
                        out=m_sb, in_=mask[qt * P : (qt + 1) * P, :]
                    )
                    s_sb = spool.tile([P, S], F32, name="scores")
                    for c in range(S // 512):
                        ps = psS.tile([P, 512], F32, name="score_ps")
                        for o in range(DO):
                            nc.tensor.matmul(
                                ps,
                                lhsT=(q_sb[:, o, :]),
                                rhs=(kt_sb[:, o, c * 512 : (c + 1) * 512]),
                                start=(o == 0),
                                stop=(o == DO - 1),
                            )
                        nc.vector.tensor_tensor(
                            s_sb[:, c * 512 : (c + 1) * 512],
                            ps,
                            m_sb[:, c * 512 : (c + 1) * 512],
                            mybir.AluOpType.add,
                        )
                    nm = stpool.tile([P, 1], F32, name="negmax")
                    nc.vector.reduce_max(
                        nm, s_sb, axis=mybir.AxisListType.X, negate=True
                    )
                    lsum = stpool.tile([P, 1], F32, name="denom")
                    nc.scalar.activation(
                        s_sb, s_sb, mybir.ActivationFunctionType.Exp,
                        bias=nm, scale=1.0, accum_out=lsum,
                    )
                    rinv = stpool.tile([P, 1], F32, name="rdenom")
                    nc.vector.reciprocal(rinv, lsum)

                    pt_sb = ptpool.tile([P, NT, P], F32R, name="probsT")
                    for kt in range(NT):
                        pst = psT.tile([P, P], F32, name="tr_ps")
                        nc.tensor.transpose(
                            pst, s_sb[:, kt * P : (kt + 1) * P], ident
                        )
                        nc.vector.tensor_copy(pt_sb[:, kt, :], pst)

                    o_sb = aopool.tile([P, DOUT], F32, name="attn_out")
                    for dh in range(DOUT // 512):
                        ps = psO.tile([P, 512], F32, name="out_ps")
                        for kt in range(NT):
                            nc.tensor.matmul(
                                ps,
                                lhsT=(pt_sb[:, kt, :]),
                                rhs=(v_sb[:, kt, dh * 512 : (dh + 1) * 512]),
                                start=(kt == 0),
                                stop=(kt == NT - 1),
                            )
                        nc.scalar.mul(o_sb[:, dh * 512 : (dh + 1) * 512], ps, rinv)
                    nc.sync.dma_start(
                        out=out[qt * P : (qt + 1) * P, :], in_=o_sb
                    )
    nc.finalize()
    return nc


def _get_nc():
    if "nc" not in _NC_CACHE:
        _NC_CACHE["nc"] = _build_bass()
    return _NC_CACHE["nc"]


def _causal_mask_half(h):
    q_idx = np.arange(QSH, dtype=np.int64)[:, None] + h * QSH
    k_idx = np.arange(S, dtype=np.int64)[None, :]
    return np.where(k_idx > q_idx, np.float32(NEG), np.float32(0.0)).astype(
        np.float32
    )


def kernel(
    inputs_for_keys,
    inputs_for_values,
    inputs_for_queries,
    WK,
    WV,
    WQ,
    _trace=False,
):
    xk = np.asarray(inputs_for_keys, dtype=np.float32)
    xv = np.asarray(inputs_for_values, dtype=np.float32)
    xq = np.asarray(inputs_for_queries, dtype=np.float32)
    wk = np.ascontiguousarray(np.asarray(WK, dtype=np.float32))
    wv = np.ascontiguousarray(np.asarray(WV, dtype=np.float32))
    wq = np.ascontiguousarray(np.asarray(WQ, dtype=np.float32))

    masks = {h: _causal_mask_half(h) for h in (0, 1)}
    xkT = [np.ascontiguousarray(xk[b].T) for b in range(B)]
    xvT = [np.ascontiguousarray(xv[b].T) for b in range(B)]

    in_maps = []
    for i in range(8):
        b, h = i // 2, i % 2
        in_maps.append(
            {
                "xqT": np.ascontiguousarray(xq[b, h * QSH : (h + 1) * QSH, :].T),
                "xkT": xkT[b],
                "xvT": xvT[b],
                "wq": wq,
                "wk": wk,
                "wv": wv,
                "mask": masks[h],
            }
        )

    nc = _get_nc()
    res = run_bass_kernel_spmd(nc, in_maps, list(range(8)), trace=_trace)

    out = np.empty((B, 2 * QSH, DOUT), dtype=np.float32)
    for i in range(8):
        b, h = i // 2, i % 2
        out[b, h * QSH : (h + 1) * QSH, :] = res.results[i]["out"]
    if _trace:
        return out, res
    return out


# revision 7
# speedup vs baseline: 1.0939x; 1.0939x over previous
"""Trainium2 Bass kernel for single-head causal attention (final).

v7's layout (transposed softmax S^T, no PE transposes, no max-subtraction,
interleaved-key causal skip, host flash-combine) plus the WQ-folding trick:

    S^T = K @ Q^T = K @ (Xq WQ)^T = (K WQ^T) @ Xq^T

Each core computes KQ^T = WQ K^T   [din, k-half]  (scaled by 1/sqrt(d))
for its own key-half only — replacing the full-sequence Q projection that
both cores of a batch used to duplicate.  The score matmuls then contract
raw Xq^T streamed from DRAM.  This halves the Q-path FLOPs with no
cross-core communication.

Per-core phases:  K proj -> KQ^T -> V proj -> attention (big query groups
first so the Xq^T prefetch stays ahead of the small groups).
Outputs: ohat [2048, 1024] (unnormalized), l [1, 2048]; the host combines
out = (ohat0 + ohat1) / (l0 + l1).
"""

import numpy as np

import concourse.bass as bass
from concourse import bacc
import concourse.mybir as mybir
import concourse.tile as tile
from concourse.bass_utils import run_bass_kernel_spmd

P = 128
B, S, DIN, DOUT = 4, 2048, 1024, 1024
KSH = S // 2        # key rows per core
KO = DIN // P       # 8 contraction sub-tiles for the projections
DO = DOUT // P      # 8 dout sub-tiles
NT = KSH // P       # 8 key tiles per core
QG = 512            # query-group width (psum free dim)
NG = S // QG        # 4 query groups
F32 = mybir.dt.float32
F32R = mybir.dt.float32r
SCALE = 1.0 / float(np.sqrt(DOUT))
NEG = -1.0e9

_NC_CACHE = {}


def _load_sliced(nc, dst, src, width, nslice, first_only=False, rest_only=False):
    """DMA a [128, KO, width] tensor in dout-slices for early availability."""
    step = width // nslice
    slices = range(nslice)
    if first_only:
        slices = range(1)
    elif rest_only:
        slices = range(1, nslice)
    for s in slices:
        nc.sync.dma_start(
            out=dst[:, :, s * step : (s + 1) * step],
            in_=src[:, :, s * step : (s + 1) * step],
        )


def _stream_projection(
    nc, pools, x3, w3, consume, seq, chunk, lhs_from_x, after_first_dma=None
):
    xpool, pspool = pools
    for c in range(seq // chunk):
        x_sb = xpool.tile([P, KO, chunk], F32R, name="xstream")
        for o in range(KO):
            nc.sync.dma_start(
                out=x_sb[:, o, :], in_=x3[:, o, c * chunk : (c + 1) * chunk]
            )
        if c == 0 and after_first_dma is not None:
            after_first_dma()
        if lhs_from_x:
            for t in range(chunk // P):
                for dh in range(DOUT // 512):
                    ps = pspool.tile([P, 512], F32, name="proj_ps")
                    for k in range(KO):
                        nc.tensor.matmul(
                            ps,
                            lhsT=x_sb[:, k, t * P : (t + 1) * P],
                            rhs=w3[:, k, dh * 512 : (dh + 1) * 512],
                            start=(k == 0),
                            stop=(k == KO - 1),
                        )
                    consume(ps, c * chunk + t * P, dh * 512)
        else:
            for o in range(DO):
                ps = pspool.tile([P, chunk], F32, name="proj_ps")
                for k in range(KO):
                    nc.tensor.matmul(
                        ps,
                        lhsT=w3[:, k, o * P : (o + 1) * P],
                        rhs=x_sb[:, k, :],
                        start=(k == 0),
                        stop=(k == KO - 1),
                    )
                consume(ps, o, c * chunk)


def _build_bass():
    nc = bacc.Bacc()
    xqT = nc.declare_dram_parameter("xqT", [DIN, S], F32R, isOutput=False)
    xkT = nc.declare_dram_parameter("xkT", [DIN, KSH], F32R, isOutput=False)
    xvT = nc.declare_dram_parameter("xvT", [DIN, KSH], F32R, isOutput=False)
    wqT = nc.declare_dram_parameter("wqT", [DOUT, DIN], F32R, isOutput=False)
    wk = nc.declare_dram_parameter("wk", [DIN, DOUT], F32R, isOutput=False)
    wv = nc.declare_dram_parameter("wv", [DIN, DOUT], F32R, isOutput=False)
    maskT = nc.declare_dram_parameter("maskT", [KSH, S], F32, isOutput=False)
    ohat = nc.declare_dram_parameter("ohat", [S, DOUT], F32, isOutput=True)
    l_out = nc.declare_dram_parameter("l", [1, S], F32, isOutput=True)

    xq3 = xqT[:, :].rearrange("(o p) q -> p o q", p=P)
    xk3 = xkT[:, :].rearrange("(o p) s -> p o s", p=P)
    xv3 = xvT[:, :].rearrange("(o p) s -> p o s", p=P)
    wq3 = wqT[:, :].rearrange("(o p) i -> p o i", p=P)   # [dout_in, dout_out, din]
    wk3 = wk[:, :].rearrange("(o p) d -> p o d", p=P)
    wv3 = wv[:, :].rearrange("(o p) d -> p o d", p=P)

    with tile.TileContext(nc) as tc:
        with tc.tile_pool(name="persist", bufs=1) as persist:
            kt_sb = persist.tile([P, DO, KSH], F32R, name="kt")    # K^T [dout, k]
            v_sb = persist.tile([P, NT, DOUT], F32R, name="v")     # V   [k, dout]
            kqt_sb = persist.tile([P, KO, KSH], F32R, name="kqt")  # KQ^T [din, k]

            # ---- Phase K: K^T for this core's key blocks
            with (
                tc.tile_pool(name="wpool_k", bufs=1) as wpool,
                tc.tile_pool(name="xs_k", bufs=2) as xpool,
                tc.tile_pool(name="ps_k", bufs=4, space="PSUM") as pspool,
            ):
                w_sb = wpool.tile([P, KO, DOUT], F32R, name="w")
                _load_sliced(nc, w_sb, wk3, DOUT, 8, first_only=True)

                def consume_k(ps, o, j):
                    nc.vector.tensor_copy(kt_sb[:, o, j : j + 512], ps)

                _stream_projection(
                    nc, (xpool, pspool), xk3, w_sb, consume_k,
                    seq=KSH, chunk=512, lhs_from_x=False,
                    after_first_dma=lambda: _load_sliced(
                        nc, w_sb, wk3, DOUT, 8, rest_only=True
                    ),
                )

            # ---- Phase KQ: KQ^T = WQ @ K^T (scaled) for this key-half
            with (
                tc.tile_pool(name="wpool_q", bufs=1) as wpool,
                tc.tile_pool(name="ps_kq", bufs=4, space="PSUM") as pspool,
            ):
                wq_sb = wpool.tile([P, KO, DIN], F32R, name="wqt")
                _load_sliced(nc, wq_sb, wq3, DIN, 8)
                for it in range(KO):          # din tile of the output
                    for kc in range(KSH // 512):
                        ps = pspool.tile([P, 512], F32, name="kq_ps")
                        for do in range(DO):  # contraction over dout
                            nc.tensor.matmul(
                                ps,
                                lhsT=wq_sb[:, do, it * P : (it + 1) * P],
                                rhs=kt_sb[:, do, kc * 512 : (kc + 1) * 512],
                                start=(do == 0),
                                stop=(do == DO - 1),
                            )
                        nc.vector.tensor_scalar_mul(
                            kqt_sb[:, it, kc * 512 : (kc + 1) * 512], ps, SCALE
                        )

            # ---- Phase V: V for this core's key blocks
            with (
                tc.tile_pool(name="wpool_v", bufs=1) as wpool,
                tc.tile_pool(name="xs_v", bufs=4) as xpool,
                tc.tile_pool(name="ps_v", bufs=4, space="PSUM") as pspool,
            ):
                w_sb = wpool.tile([P, KO, DOUT], F32R, name="w")
                _load_sliced(nc, w_sb, wv3, DOUT, 8)

                def consume_v(ps, s0, d0):
                    nc.vector.tensor_copy(v_sb[:, s0 // P, d0 : d0 + 512], ps)

                _stream_projection(
                    nc, (xpool, pspool), xv3, w_sb, consume_v,
                    seq=KSH, chunk=256, lhs_from_x=True,
                )

            # ---- Phase A: causal-skip transposed-softmax attention.
            # Raw Xq^T streams from DRAM per query group; groups run
            # largest-first so prefetch covers the small (DMA-bound) ones.
            with (
                tc.tile_pool(name="ones", bufs=1) as onepool,
                tc.tile_pool(name="lrow", bufs=1) as lpool,
                tc.tile_pool(name="xq_g", bufs=2) as xqpool,
                tc.tile_pool(name="msk", bufs=3) as mpool,
                tc.tile_pool(name="slab", bufs=2) as slabpool,
                tc.tile_pool(name="ao", bufs=3) as aopool,
                tc.tile_pool(name="ps_s", bufs=4, space="PSUM") as psS,
                tc.tile_pool(name="ps_l", bufs=2, space="PSUM") as psL,
                tc.tile_pool(name="ps_o", bufs=2, space="PSUM") as psO,
            ):
                ones_f32 = onepool.tile([P, 1], F32, name="ones_f32")
                nc.vector.memset(ones_f32, 1.0)
                ones_sb = onepool.tile([P, 1], F32R, name="ones")
                nc.vector.tensor_copy(ones_sb, ones_f32)
                l_sb = lpool.tile([1, S], F32, name="l_row")

                for g in reversed(range(NG)):
                    lim = min(NT, 2 * g + 2)   # k-tiles actually attended
                    xq_g = xqpool.tile([P, KO, QG], F32R, name="xq_group")
                    for o in range(KO):
                        nc.sync.dma_start(
                            out=xq_g[:, o, :],
                            in_=xq3[:, o, g * QG : (g + 1) * QG],
                        )
                    slab = slabpool.tile([P, NT, QG], F32R, name="expT")
                    for kt in range(lim):
                        ps = psS.tile([P, QG], F32, name="score_ps")
                        for io in range(KO):
                            nc.tensor.matmul(
                                ps,
                                lhsT=kqt_sb[:, io, kt * P : (kt + 1) * P],
                                rhs=xq_g[:, io, :],
                                start=(io == 0),
                                stop=(io == KO - 1),
                            )
                        if kt >= 2 * g:
                            m_sb = mpool.tile([P, QG], F32, name="maskT_rows")
                            nc.sync.dma_start(
                                out=m_sb,
                                in_=maskT[
                                    kt * P : (kt + 1) * P,
                                    g * QG : (g + 1) * QG,
                                ],
                            )
                            nc.vector.tensor_tensor(
                                slab[:, kt, :], ps, m_sb, mybir.AluOpType.add
                            )
                            nc.scalar.activation(
                                slab[:, kt, :], slab[:, kt, :],
                                mybir.ActivationFunctionType.Exp,
                            )
                        else:
                            nc.scalar.activation(
                                slab[:, kt, :], ps,
                                mybir.ActivationFunctionType.Exp,
                            )

                    ps_l = psL.tile([1, QG], F32, name="l_ps")
                    for kt in range(lim):
                        nc.tensor.matmul(
                            ps_l,
                            lhsT=ones_sb,
                            rhs=slab[:, kt, :],
                            start=(kt == 0),
                            stop=(kt == lim - 1),
                        )
                    nc.vector.tensor_copy(l_sb[:, g * QG : (g + 1) * QG], ps_l)

                    for t in range(QG // P):
                        o_sb = aopool.tile([P, DOUT], F32, name="attn_out")
                        for dh in range(DOUT // 512):
                            ps = psO.tile([P, 512], F32, name="out_ps")
                            for kt in range(lim):
                                nc.tensor.matmul(
                                    ps,
                                    lhsT=slab[:, kt, t * P : (t + 1) * P],
                                    rhs=v_sb[:, kt, dh * 512 : (dh + 1) * 512],
                                    start=(kt == 0),
                                    stop=(kt == lim - 1),
                                )
                            nc.scalar.copy(
                                o_sb[:, dh * 512 : (dh + 1) * 512], ps
                            )
                        q0 = g * QG + t * P
                        nc.sync.dma_start(out=ohat[q0 : q0 + P, :], in_=o_sb)

                nc.sync.dma_start(out=l_out[:, :], in_=l_sb)
    nc.finalize()
    return nc


def _get_nc():
    if "nc" not in _NC_CACHE:
        _NC_CACHE["nc"] = _build_bass()
    return _NC_CACHE["nc"]


def _key_index(hk):
    """Global key rows owned by core hk: interleaved 128-row blocks."""
    blocks = np.arange(hk, S // P, 2)
    return (blocks[:, None] * P + np.arange(P)[None, :]).reshape(-1)


def _maskT_half(hk):
    """Additive causal mask, transposed, for the interleaved key set."""
    k_idx = _key_index(hk)[:, None]
    q_idx = np.arange(S, dtype=np.int64)[None, :]
    return np.where(k_idx > q_idx, np.float32(NEG), np.float32(0.0)).astype(
        np.float32
    )


def kernel(
    inputs_for_keys,
    inputs_for_values,
    inputs_for_queries,
    WK,
    WV,
    WQ,
    _trace=False,
):
    xk = np.asarray(inputs_for_keys, dtype=np.float32)
    xv = np.asarray(inputs_for_values, dtype=np.float32)
    xq = np.asarray(inputs_for_queries, dtype=np.float32)
    wk = np.ascontiguousarray(np.asarray(WK, dtype=np.float32))
    wv = np.ascontiguousarray(np.asarray(WV, dtype=np.float32))
    wqT = np.ascontiguousarray(np.asarray(WQ, dtype=np.float32).T)

    masks = {hk: _maskT_half(hk) for hk in (0, 1)}
    kidx = {hk: _key_index(hk) for hk in (0, 1)}
    xqTb = [np.ascontiguousarray(xq[b].T) for b in range(B)]

    in_maps = []
    for i in range(8):
        b, hk = i // 2, i % 2
        in_maps.append(
            {
                "xqT": xqTb[b],
                "xkT": np.ascontiguousarray(xk[b][kidx[hk]].T),
                "xvT": np.ascontiguousarray(xv[b][kidx[hk]].T),
                "wqT": wqT,
                "wk": wk,
                "wv": wv,
                "maskT": masks[hk],
            }
        )

    nc = _get_nc()
    res = run_bass_kernel_spmd(nc, in_maps, list(range(8)), trace=_trace)

    out = np.empty((B, S, DOUT), dtype=np.float32)
    for b in range(B):
        r0 = res.results[2 * b]
        r1 = res.results[2 * b + 1]
        den = (r0["l"] + r1["l"]).reshape(S, 1)
        out[b] = (r0["ohat"] + r1["ohat"]) / den
    if _trace:
        return out, res
    return out
